# revision 2
# baseline (speedup 1.0000x reference)
"""Trainium2 Bass kernel for a single transformer encoder layer (v6).

Problem: B=4, S=2048, D=512, H=8 (dk=64), DFF=2048, f32 I/O.
Sharding: 8 cores = (batch b, token-half). Each core computes the full
layer for its 1024 tokens; K/V are computed for the whole 2048-token
context on both cores of a pair (duplicated, zero communication).

History: v1 (bf16, coarse bursts, 315us). v2-v5 (fp8 + fine-grained
quanta interleave, 332-350us): the many sub-us PE bubbles between
interleaved slivers depressed PE-array duty below the HAM clock-gate
threshold (throttle 119-167us at half clock), erasing every theoretical
win. Lesson: keep PE work in DENSE BURSTS; the softmax exp spine (ACT)
has slack and absorbs the burst delays.

v6 = v1's burst schedule + surgical wins:
  - fp8e4 inputs/weights for Q/K/V/Wo projections via DoubleRow
    (contract 512 = 2 passes, ~0.56x streaming time) -- these run in
    dense phases where HAM stays warm; ctxT stored fp8 for the Wo DR.
    1/sqrt(dk) folded into the softmax EXP free scale (NOT into the
    fp8 wq, which would hit subnormals). fp8 xT also halves the
    startup-critical DMA.
  - scores/ctx/FFN stay bf16 (attention duty floor keeps PE at 2.4GHz;
    fp8 FFN blows the 2e-2 error budget: sim 1.1-1.7e-2).
  - V per head with a ones column at 64 -> ctx PSUM row 64 accumulates
    the softmax denominator Z for free; 1/Z applied via ln/exp on ACT
    and ONE 2-contract broadcast matmul per head pair (ind2).
  - 3 pipelined t1-blocks (256, 512, 256): block i's post+FFN runs as
    coarse stage-bursts in TWO slots per pair (mid-pair, pair end) of
    block i+1's attention; only block C's post+FFN is an exposed tail,
    run with split LN chains (ACT ops never wait on fresh DVE stats).
  - bo folded into xo on the host; w1/w2/xo prefetches ride the gpsimd
    DMA queue AFTER the small consts (a DMA trigger blocks its issuing
    engine's FIFO while the DMA queue is full).
  - ScalarE runs ONLY Exp/Ln/Identity/Relu (one activation-table set).
"""

from contextlib import ExitStack

import numpy as np
import ml_dtypes

import concourse.bass as bass
import concourse.tile as tile
from concourse import mybir, bacc
from concourse.bass_utils import run_bass_kernel_spmd
from concourse.masks import make_identity

F32 = mybir.dt.float32
BF16 = mybir.dt.bfloat16
F8 = mybir.dt.float8e4
AF = mybir.ActivationFunctionType
OP = mybir.AluOpType
DR = mybir.MatmulPerfMode.DoubleRow

B, S, D = 4, 2048, 512
H, DK, DFF = 8, 64, 2048
EPS = 1e-5
P = 128
T1 = 1024          # own tokens per core
NCORES = 8

KD = D // P        # 4   feature k-tiles
NT2 = S // P       # 16  t2 tiles (context tokens)
NT1 = T1 // P      # 8   own-token 128-tiles
NPAIR = H // 2     # 4   head pairs
NDFF = DFF // P    # 16  dff tiles
DV1 = DK + 1       # 65  V columns incl the ones column
SCALE = 1.0 / np.sqrt(DK)

# t1 pipeline blocks: (offset, width)
BLOCKS = [(0, 256), (256, 512), (768, 256)]


def emit(ctx: ExitStack, tc, io):
    nc = tc.nc

    xT, xTo, xo = io["xT"], io["xTo"], io["xo"]
    wq, wk, wv, wo, w1, w2 = io["wq"], io["wk"], io["wv"], io["wo"], io["w1"], io["w2"]
    out = io["out"]

    const = ctx.enter_context(tc.tile_pool(name="const", bufs=1))
    persist = ctx.enter_context(tc.tile_pool(name="persist", bufs=1))
    exp_pool = ctx.enter_context(tc.tile_pool(name="exp", bufs=3))
    cxu_pool = ctx.enter_context(tc.tile_pool(name="cxu", bufs=6))
    work = ctx.enter_context(tc.tile_pool(name="work", bufs=2))
    stat = ctx.enter_context(tc.tile_pool(name="stat", bufs=4))
    norm = ctx.enter_context(tc.tile_pool(name="norm", bufs=2))
    zpool = ctx.enter_context(tc.tile_pool(name="zpool", bufs=1))
    out_pool = ctx.enter_context(tc.tile_pool(name="out", bufs=2))

    mm_ps = ctx.enter_context(tc.tile_pool(name="mm_ps", bufs=2, space="PSUM"))
    sc_ps = ctx.enter_context(tc.tile_pool(name="sc_ps", bufs=2, space="PSUM"))
    ctx_ps = ctx.enter_context(tc.tile_pool(name="ctx_ps", bufs=1, space="PSUM"))

    # ---- persistent SBUF arrays ----
    wk_sb = persist.tile([P, KD, D], F8, tag="wk")
    nc.sync.dma_start(wk_sb[:], wk[:, :].rearrange("(k p) m -> p k m", p=P))
    xT_sb = persist.tile([P, KD, S], F8, tag="xT")
    for k in range(KD):
        nc.sync.dma_start(
            xT_sb[:, k, :], xT[:, :].rearrange("(k p) t -> p k t", p=P)[:, k, :]
        )
    wq_sb = persist.tile([P, KD, D], F8, tag="wq")
    nc.sync.dma_start(wq_sb[:], wq[:, :].rearrange("(k p) m -> p k m", p=P))
    xTo_sb = persist.tile([P, KD, T1], F8, tag="xTo")
    for k in range(KD):
        nc.sync.dma_start(
            xTo_sb[:, k, :], xTo[:, :].rearrange("(k p) t -> p k t", p=P)[:, k, :]
        )
    wv_sb = persist.tile([P, KD, D], F8, tag="wv")
    nc.sync.dma_start(wv_sb[:], wv[:, :].rearrange("(k p) m -> p k m", p=P))
    wo_sb = persist.tile([P, KD, D], F8, tag="wo")
    nc.sync.dma_start(wo_sb[:], wo[:, :].rearrange("(k p) m -> p k m", p=P))

    kt_sb = persist.tile([P, NPAIR, S], BF16, tag="kt")
    qt_sb = persist.tile([P, NPAIR, T1], BF16, tag="qt")
    # V per head with ones column at 64: [t2 128, t2tile, head, 65] (bf16)
    ve_sb = persist.tile([P, NT2, H, DV1], BF16, tag="ve")
    nc.vector.memset(ve_sb[:, :, :, DK:DV1], 1.0)
    ctxT_sb = persist.tile([P, NPAIR, T1], F8, tag="ctxT")
    x1_sb = persist.tile([P, NT1, D], BF16, tag="x1")
    x1T_sb = persist.tile([P, KD, T1], BF16, tag="x1T")
    h1T_sb = persist.tile([P, NDFF, T1], BF16, tag="h1T")
    w1_sb = persist.tile([P, KD, DFF], BF16, tag="w1")
    w2_sb = persist.tile([P, NDFF, D], BF16, tag="w2")
    xo_sb = persist.tile([P, NT1, D], BF16, tag="xo")

    # ---- constants ----
    ident_sb = const.tile([P, P], BF16)
    make_identity(nc, ident_sb[:])
    eps_sb = const.tile([P, 1], F32)
    nc.vector.memset(eps_sb[:], EPS)
    # indicator for the 1/Z partition-broadcast: row 0 -> partitions 0:64
    # (even head), row 1 -> partitions 64:128 (odd head)
    ind2_sb = const.tile([2, P], BF16)
    nc.gpsimd.dma_start(ind2_sb[:], io["ind2"][:, :])

    # per-partition bias tiles (feature-major evictions)
    bqt = const.tile([P, KD], F32)
    nc.gpsimd.dma_start(bqt[:], io["bq"][:].rearrange("(m p) -> p m", p=P))
    bkt = const.tile([P, KD], F32)
    nc.gpsimd.dma_start(bkt[:], io["bk"][:].rearrange("(m p) -> p m", p=P))
    b1t = const.tile([P, NDFF], F32)
    nc.gpsimd.dma_start(b1t[:], io["b1"][:].rearrange("(m p) -> p m", p=P))

    # free-axis broadcast tiles (token-major ops)
    def bc_tile(name):
        t = const.tile([P, D], BF16, tag=f"bc_{name}")
        a = io[name][:]
        bcast = bass.AP(tensor=a.tensor, offset=a.offset, ap=[[0, P]] + list(a.ap))
        nc.gpsimd.dma_start(t[:], bcast)
        return t

    bvb = bc_tile("bv")
    b2b = bc_tile("b2")
    g1b = bc_tile("g1")
    be1b = bc_tile("be1")
    g2b = bc_tile("g2")
    be2b = bc_tile("be2")

    # big prefetches ride the (otherwise idle) gpsimd DMA queue, AFTER the
    # consts above -- a DMA trigger blocks its issuing engine's FIFO while
    # the queue is full.
    nc.gpsimd.dma_start(w1_sb[:], w1[:, :].rearrange("(k p) m -> p k m", p=P))
    nc.gpsimd.dma_start(w2_sb[:], w2[:, :].rearrange("(k p) m -> p k m", p=P))
    # residual (token-major, bf16, bo pre-added on host) preloaded once
    for t in range(NT1):
        nc.gpsimd.dma_start(xo_sb[:, t, :], xo[t * P:(t + 1) * P, :])

    # ---- projections (fp8 DoubleRow, contract 512 = 2 passes) ----
    def kproj(m, nb):
        ps = mm_ps.tile([P, 512], F32, tag="mm")
        for j in range(2):
            nc.tensor.matmul(
                ps[:],
                wk_sb[:, 2 * j:2 * j + 2, m * P:(m + 1) * P],
                xT_sb[:, 2 * j:2 * j + 2, nb * 512:(nb + 1) * 512],
                start=(j == 0), stop=(j == 1), perf_mode=DR,
            )
        nc.scalar.activation(
            kt_sb[:, m, nb * 512:(nb + 1) * 512], ps[:], AF.Identity,
            bias=bkt[:, m:m + 1],
        )

    def qproj(m, nb):
        ps = mm_ps.tile([P, 512], F32, tag="mm")
        for j in range(2):
            nc.tensor.matmul(
                ps[:],
                wq_sb[:, 2 * j:2 * j + 2, m * P:(m + 1) * P],
                xTo_sb[:, 2 * j:2 * j + 2, nb * 512:(nb + 1) * 512],
                start=(j == 0), stop=(j == 1), perf_mode=DR,
            )
        nc.scalar.activation(
            qt_sb[:, m, nb * 512:(nb + 1) * 512], ps[:], AF.Identity,
            bias=bqt[:, m:m + 1],
        )

    def vproj(i):
        ps = mm_ps.tile([P, 512], F32, tag="mm")
        for j in range(2):
            nc.tensor.matmul(
                ps[:],
                xT_sb[:, 2 * j:2 * j + 2, i * P:(i + 1) * P],
                wv_sb[:, 2 * j:2 * j + 2, :],
                start=(j == 0), stop=(j == 1), perf_mode=DR,
            )
        nc.vector.tensor_tensor(
            ve_sb[:, i, :, 0:DK],
            ps[:].rearrange("p (h d) -> p h d", h=H),
            bvb[:].rearrange("p (h d) -> p h d", h=H),
            OP.add,
        )

    # ---- post-attention / FFN stages ----
    post_stats = {}

    def post_attn1(t1t, eng):
        """Wo (fp8 DR) + residual (bo pre-folded into xo) + bn stats."""
        ao = mm_ps.tile([P, 512], F32, tag="mm")
        for j in range(2):
            nc.tensor.matmul(
                ao[:],
                ctxT_sb[:, 2 * j:2 * j + 2, t1t * P:(t1t + 1) * P],
                wo_sb[:, 2 * j:2 * j + 2, :],
                start=(j == 0), stop=(j == 1), perf_mode=DR,
            )
        rslot = x1_sb[:, t1t, :]
        nc.vector.tensor_tensor(rslot, ao[:], xo_sb[:, t1t, :], OP.add)
        st = stat.tile([P, 6], F32, tag="st")
        nc.vector.bn_stats(st[:], rslot)
        mv = stat.tile([P, 2], F32, tag="mv")
        nc.vector.bn_aggr(mv[:], st[:])
        post_stats[t1t] = [mv, None]

    def post_rstd(t1t):
        """LN1 rstd on ACT (its DVE inputs are a stage old)."""
        mv = post_stats[t1t][0]
        lnv = stat.tile([P, 1], F32, tag="lnv")
        nc.scalar.activation(lnv[:], mv[:, 1:2], AF.Ln, bias=eps_sb[:, 0:1])
        rstd = stat.tile([P, 1], F32, tag="rstd")
        nc.scalar.activation(rstd[:], lnv[:], AF.Exp, scale=-0.5)
        post_stats[t1t][1] = rstd

    def post_attn2(t1t, eng):
        """LN1 normalize+affine (in the x1 slot) + transpose(x1) -> x1T."""
        mv, rstd = post_stats.pop(t1t)
        rslot = x1_sb[:, t1t, :]
        xc = work.tile([P, D], F32, tag="xc")
        nc.vector.tensor_scalar(
            xc[:], rslot, mv[:, 0:1], rstd[:], op0=OP.subtract, op1=OP.mult
        )
        xg = work.tile([P, D], F32, tag="xg")
        eng.tensor_tensor(xg[:], xc[:], g1b[:], OP.mult)
        eng.tensor_tensor(rslot, xg[:], be1b[:], OP.add)
        for j in range(KD):
            tp = mm_ps.tile([P, P], BF16, tag="mm")
            nc.tensor.transpose(
                tp[:], x1_sb[:, t1t, j * P:(j + 1) * P], ident_sb[:]
            )
            if eng is nc.vector:
                nc.scalar.copy(x1T_sb[:, j, t1t * P:(t1t + 1) * P], tp[:])
            else:
                nc.vector.tensor_copy(x1T_sb[:, j, t1t * P:(t1t + 1) * P], tp[:])

    def ffn1(lo, tb, m0, m1, on_act):
        for m in range(m0, m1):
            ps = mm_ps.tile([P, 512], F32, tag="mm")
            for k in range(KD):
                nc.tensor.matmul(
                    ps[:, 0:tb],
                    w1_sb[:, k, m * P:(m + 1) * P],
                    x1T_sb[:, k, lo:lo + tb],
                    start=(k == 0), stop=(k == KD - 1),
                )
            if on_act:
                nc.scalar.activation(
                    h1T_sb[:, m, lo:lo + tb], ps[:, 0:tb], AF.Relu,
                    bias=b1t[:, m:m + 1],
                )
            else:
                nc.vector.tensor_scalar(
                    h1T_sb[:, m, lo:lo + tb], ps[:, 0:tb],
                    b1t[:, m:m + 1], 0.0, op0=OP.add, op1=OP.max,
                )

    ffn2_stats = {}

    def ffn2a(t1t, eng):
        """FFN2 matmuls + residual + LN2 stats (no ACT ops)."""
        ff = mm_ps.tile([P, 512], F32, tag="mm")
        for k in range(NDFF):
            nc.tensor.matmul(
                ff[:],
                h1T_sb[:, k, t1t * P:(t1t + 1) * P],
                w2_sb[:, k, :],
                start=(k == 0), stop=(k == NDFF - 1),
            )
        r = work.tile([P, D], F32, tag="r2")
        nc.vector.tensor_tensor(r[:], ff[:], x1_sb[:, t1t, :], OP.add)
        nc.vector.tensor_tensor(r[:], r[:], b2b[:], OP.add)
        st = stat.tile([P, 6], F32, tag="st")
        nc.vector.bn_stats(st[:], r[:])
        mv = stat.tile([P, 2], F32, tag="mv")
        nc.vector.bn_aggr(mv[:], st[:])
        ffn2_stats[t1t] = (r, mv)

    def ffn2b(t1t, eng):
        """LN2 rstd (ACT; stats a stage old) + normalize + store."""
        r, mv = ffn2_stats.pop(t1t)
        lnv = stat.tile([P, 1], F32, tag="lnv")
        nc.scalar.activation(lnv[:], mv[:, 1:2], AF.Ln, bias=eps_sb[:, 0:1])
        rstd = stat.tile([P, 1], F32, tag="rstd")
        nc.scalar.activation(rstd[:], lnv[:], AF.Exp, scale=-0.5)
        xc = work.tile([P, D], F32, tag="xc")
        nc.vector.tensor_scalar(
            xc[:], r[:], mv[:, 0:1], rstd[:], op0=OP.subtract, op1=OP.mult
        )
        o = out_pool.tile([P, D], F32)
        xg = work.tile([P, D], F32, tag="xg")
        eng.tensor_tensor(xg[:], xc[:], g2b[:], OP.mult)
        eng.tensor_tensor(o[:], xg[:], be2b[:], OP.add)
        nc.sync.dma_start(out[t1t * P:(t1t + 1) * P, :], o[:])

    # ---- attention ----
    def normalize_pair(pair, t1s, tb, zall, cxu):
        """1/Z for one head pair: [2,tb] ln/exp + ONE 2-contract broadcast
        matmul (even head -> partitions 0:64, odd -> 64:128)."""
        hA, hB = 2 * pair, 2 * pair + 1
        zs = slice(pair * 512, pair * 512 + tb)
        lz = norm.tile([2, 512], F32, tag="lz")
        nc.scalar.activation(lz[:, 0:tb], zall[0:2, zs], AF.Ln)
        rz = norm.tile([2, 512], BF16, tag="rz")
        nc.scalar.activation(rz[:, 0:tb], lz[:, 0:tb], AF.Exp, scale=-1.0)
        bch = mm_ps.tile([P, 512], F32, tag="mm")
        nc.tensor.matmul(
            bch[:, 0:tb], ind2_sb[:, :], rz[:, 0:tb], start=True, stop=True,
        )
        nc.vector.tensor_tensor(
            ctxT_sb[0:64, pair, t1s], cxu.pop(hA)[:, 0:tb], bch[0:64, 0:tb],
            OP.mult,
        )
        stg = work.tile([64, 512], F8, tag="stg")
        nc.vector.tensor_tensor(
            stg[:, 0:tb], cxu.pop(hB)[:, 0:tb], bch[64:128, 0:tb], OP.mult
        )
        nc.sync.dma_start(ctxT_sb[64:128, pair, t1s], stg[:, 0:tb])

    def attention_block(b, slot):
        """slot(b, pair, half) is called twice per pair (mid-pair, pair
        end) to emit one hidden-work stage as a dense burst."""
        lo, tb = BLOCKS[b]
        t1s = slice(lo, lo + tb)
        zall = zpool.tile([2, NPAIR * 512], F32, tag="zall")
        cxu = {}
        for pair in range(NPAIR):
            hA, hB = 2 * pair, 2 * pair + 1
            cxA = ctx_ps.tile([DV1, 512], F32, tag="cxA")
            cxB = ctx_ps.tile([DV1, 512], F32, tag="cxB")
            for t2 in range(NT2):
                if t2 == 2 and pair > 0:
                    # previous pair's Z rows have landed by now
                    normalize_pair(pair - 1, t1s, tb, zall, cxu)
                sp = sc_ps.tile([P, 2, 512], F32, tag="s")
                for idx, hrow in enumerate((0, 64)):
                    nc.tensor.matmul(
                        sp[:, idx, 0:tb],
                        kt_sb[hrow:hrow + 64, pair, t2 * P:(t2 + 1) * P],
                        qt_sb[hrow:hrow + 64, pair, t1s],
                        start=True, stop=True, tile_position=(hrow, 0),
                        skip_group_check=(idx > 0),
                    )
                e = exp_pool.tile([P, 2, 512], BF16, tag="e")
                nc.scalar.activation(
                    e[:, :, 0:tb], sp[:, :, 0:tb], AF.Exp, scale=SCALE
                )
                first, last = t2 == 0, t2 == NT2 - 1
                nc.tensor.matmul(
                    cxA[:, 0:tb], ve_sb[:, t2, hA, :], e[:, 0, 0:tb],
                    start=first, stop=last,
                )
                nc.tensor.matmul(
                    cxB[:, 0:tb], ve_sb[:, t2, hB, :], e[:, 1, 0:tb],
                    start=first, stop=last,
                )
                if t2 == NT2 // 2:
                    slot(b, pair, 0)
            # evict unnormalized ctx (bf16) and gather Z rows (f32)
            for h, cx in ((hA, cxA), (hB, cxB)):
                cu = cxu_pool.tile([64, 512], BF16, tag="cu")
                nc.vector.tensor_copy(cu[:, 0:tb], cx[0:64, 0:tb])
                zst = norm.tile([P, 512], F32, tag="zst")
                nc.vector.tensor_copy(zst[64:65, 0:tb], cx[64:65, 0:tb])
                nc.sync.dma_start(
                    zall[h & 1:(h & 1) + 1, pair * 512:pair * 512 + tb],
                    zst[64:65, 0:tb],
                )
                cxu[h] = cu
            slot(b, pair, 1)
        normalize_pair(NPAIR - 1, t1s, tb, zall, cxu)

    # hidden-work stages for a finished block, run as dense bursts in the
    # next block's 8 slots (2 per pair)
    def make_stages(b):
        lo, tb = BLOCKS[b]
        hidden = b < len(BLOCKS) - 1
        eng = nc.gpsimd if hidden else nc.vector
        tiles = [lo // P + i for i in range(tb // P)]
        half = (len(tiles) + 1) // 2

        def s0():
            for t in tiles[:half]:
                post_attn1(t, eng)

        def s1():
            for t in tiles[half:]:
                post_attn1(t, eng)
            for t in tiles:
                post_rstd(t)

        def s2():
            for t in tiles[:half]:
                post_attn2(t, eng)

        def s3():
            for t in tiles[half:]:
                post_attn2(t, eng)

        def s4():
            ffn1(lo, tb, 0, NDFF // 2, on_act=not hidden)

        def s5():
            ffn1(lo, tb, NDFF // 2, NDFF, on_act=not hidden)

        def s6():
            # a/b paired two tiles at a time: keeps <=2 r tiles in flight
            # (work pool bufs=2; more would WAR-deadlock the DVE FIFO)
            for t in tiles[:2]:
                ffn2a(t, eng)
            for t in tiles[:2]:
                ffn2b(t, eng)

        def s7():
            for t in tiles[2:]:
                ffn2a(t, eng)
            for t in tiles[2:]:
                ffn2b(t, eng)

        return [s0, s1, s2, s3, s4, s5, s6, s7]

    pending = {}

    def slot(b, pair, half):
        idx = 2 * pair + half
        stages = pending.get(b - 1)
        if stages and idx < len(stages):
            stages[idx]()

    # startup: all projections upfront as one dense burst (fp8 DMA is
    # cheap; PE warms up on them)
    for m in range(NPAIR):
        for nb in range(4):
            kproj(m, nb)
    for m in range(NPAIR):
        for nb in range(2):
            qproj(m, nb)
    for i in range(NT2):
        vproj(i)

    for b in range(len(BLOCKS)):
        attention_block(b, slot)
        pending[b] = make_stages(b)
    # tail: last block's stages, plus anything unfinished
    for st in pending[len(BLOCKS) - 1]:
        st()


def _patch_act_tables():
    """Force every ACT op onto the natural_log_exp_and_others table set so
    the kernel pays one ACT_TABLE_LOAD instead of thrashing between the
    per-function default sets."""
    import functools
    import concourse.hw_specs as hw_specs

    if getattr(hw_specs, "_nle_only", False):
        return
    orig = hw_specs.get_activation_tables

    @functools.cache
    def nle_only(arch):
        tabs = orig(arch)
        return {
            k: (v if k == "natural_log_exp_and_others" else set())
            for k, v in tabs.items()
        }

    hw_specs.get_activation_tables = nle_only
    hw_specs._nle_only = True
    if getattr(bacc, "get_activation_tables", None) is not None:
        bacc.get_activation_tables = nle_only


def build_program():
    _patch_act_tables()
    nc = bacc.Bacc("TRN2", target_bir_lowering=False, debug=False, num_devices=NCORES)
    io = {}
    io["xT"] = nc.dram_tensor("xT", [D, S], F8, kind="ExternalInput").ap()
    io["xTo"] = nc.dram_tensor("xTo", [D, T1], F8, kind="ExternalInput").ap()
    io["xo"] = nc.dram_tensor("xo", [T1, D], BF16, kind="ExternalInput").ap()
    for name, shape, dt in [
        ("wq", [D, D], F8), ("wk", [D, D], F8), ("wv", [D, D], F8),
        ("wo", [D, D], F8), ("w1", [D, DFF], BF16), ("w2", [DFF, D], BF16),
    ]:
        io[name] = nc.dram_tensor(name, shape, dt, kind="ExternalInput").ap()
    for name, n in [
        ("bq", D), ("bk", D), ("bv", D), ("bo", D), ("b1", DFF), ("b2", D),
        ("g1", D), ("be1", D), ("g2", D), ("be2", D),
    ]:
        io[name] = nc.dram_tensor(name, [n], F32, kind="ExternalInput").ap()
    io["ind2"] = nc.dram_tensor("ind2", [2, P], BF16, kind="ExternalInput").ap()
    io["out"] = nc.dram_tensor("out", [T1, D], F32, kind="ExternalOutput").ap()

    with tile.TileContext(nc) as tc:
        with ExitStack() as ctx:
            emit(ctx, tc, io)
    nc.compile()
    return nc


def make_in_maps(x, Wq, bq, Wk, bk, Wv, bv, Wo, bo, W1, b1, W2, b2,
                 g1, be1, g2, be2):
    bf = ml_dtypes.bfloat16
    f8 = ml_dtypes.float8_e4m3fn
    f32 = np.float32
    shared = {
        "wq": np.asarray(Wq, f32).astype(f8),
        "wk": np.asarray(Wk, f32).astype(f8),
        "wv": np.asarray(Wv, f32).astype(f8),
        "wo": np.asarray(Wo, f32).astype(f8),
        "w1": np.asarray(W1, f32).astype(bf),
        "w2": np.asarray(W2, f32).astype(bf),
        "bq": np.asarray(bq, f32),
        "bk": np.asarray(bk, f32), "bv": np.asarray(bv, f32),
        "bo": np.asarray(bo, f32), "b1": np.asarray(b1, f32),
        "b2": np.asarray(b2, f32), "g1": np.asarray(g1, f32),
        "be1": np.asarray(be1, f32), "g2": np.asarray(g2, f32),
        "be2": np.asarray(be2, f32),
        "ind2": np.kron(np.eye(2, dtype=f32), np.ones((1, DK), f32)).astype(bf),
    }
    x = np.asarray(x, f32)
    in_maps = []
    for c in range(NCORES):
        b, half = divmod(c, 2)
        xb = x[b]                                    # [S, D] f32
        xTb = np.ascontiguousarray(xb.T).astype(f8)  # [D, S] fp8
        sl = slice(half * T1, (half + 1) * T1)
        m = dict(shared)
        m["xT"] = xTb
        m["xTo"] = np.ascontiguousarray(xTb[:, sl])
        # bo folded into the residual on the host
        m["xo"] = (xb[sl] + np.asarray(bo, f32)[None, :]).astype(bf)
        in_maps.append(m)
    return in_maps


_prog_cache = {}


def get_program():
    if "nc" not in _prog_cache:
        _prog_cache["nc"] = build_program()
    return _prog_cache["nc"]


def kernel(**inputs) -> np.ndarray:
    nc = get_program()
    in_maps = make_in_maps(**inputs)
    res = run_bass_kernel_spmd(nc, in_maps, core_ids=list(range(NCORES)))
    out = np.empty((B, S, D), np.float32)
    for c in range(NCORES):
        b, half = divmod(c, 2)
        out[b, half * T1:(half + 1) * T1] = res.results[c]["out"]
    return out


if __name__ == "__main__":
    print("building program...")
    get_program()
    print("built")


# revision 3
# speedup vs baseline: 1.0080x; 1.0080x over previous
"""Trainium2 Bass kernel for a single transformer encoder layer (v6).

Problem: B=4, S=2048, D=512, H=8 (dk=64), DFF=2048, f32 I/O.
Sharding: 8 cores = (batch b, token-half). Each core computes the full
layer for its 1024 tokens; K/V are computed for the whole 2048-token
context on both cores of a pair (duplicated, zero communication).

History: v1 (bf16, coarse bursts, 315us). v2-v5 (fp8 + fine-grained
quanta interleave, 332-350us): the many sub-us PE bubbles between
interleaved slivers depressed PE-array duty below the HAM clock-gate
threshold (throttle 119-167us at half clock), erasing every theoretical
win. Lesson: keep PE work in DENSE BURSTS; the softmax exp spine (ACT)
has slack and absorbs the burst delays.

v6 = v1's burst schedule + surgical wins:
  - fp8e4 inputs/weights for Q/K/V/Wo projections via DoubleRow
    (contract 512 = 2 passes, ~0.56x streaming time) -- these run in
    dense phases where HAM stays warm; ctxT stored fp8 for the Wo DR.
    1/sqrt(dk) folded into the softmax EXP free scale (NOT into the
    fp8 wq, which would hit subnormals). fp8 xT also halves the
    startup-critical DMA.
  - scores/ctx/FFN stay bf16 (attention duty floor keeps PE at 2.4GHz;
    fp8 FFN blows the 2e-2 error budget: sim 1.1-1.7e-2).
  - V per head with a ones column at 64 -> ctx PSUM row 64 accumulates
    the softmax denominator Z for free; 1/Z applied via ln/exp on ACT
    and ONE 2-contract broadcast matmul per head pair (ind2).
  - 3 pipelined t1-blocks (256, 512, 256): block i's post+FFN runs as
    coarse stage-bursts in TWO slots per pair (mid-pair, pair end) of
    block i+1's attention; only block C's post+FFN is an exposed tail,
    run with split LN chains (ACT ops never wait on fresh DVE stats).
  - bo folded into xo on the host; w1/w2/xo prefetches ride the gpsimd
    DMA queue AFTER the small consts (a DMA trigger blocks its issuing
    engine's FIFO while the DMA queue is full).
  - ScalarE runs ONLY Exp/Ln/Identity/Relu (one activation-table set).
"""

from contextlib import ExitStack

import numpy as np
import ml_dtypes

import concourse.bass as bass
import concourse.tile as tile
from concourse import mybir, bacc
from concourse.bass_utils import run_bass_kernel_spmd
from concourse.masks import make_identity

F32 = mybir.dt.float32
BF16 = mybir.dt.bfloat16
F8 = mybir.dt.float8e4
AF = mybir.ActivationFunctionType
OP = mybir.AluOpType
DR = mybir.MatmulPerfMode.DoubleRow

B, S, D = 4, 2048, 512
H, DK, DFF = 8, 64, 2048
EPS = 1e-5
P = 128
T1 = 1024          # own tokens per core
NCORES = 8

KD = D // P        # 4   feature k-tiles
NT2 = S // P       # 16  t2 tiles (context tokens)
NT1 = T1 // P      # 8   own-token 128-tiles
NPAIR = H // 2     # 4   head pairs
NDFF = DFF // P    # 16  dff tiles
DV1 = DK + 1       # 65  V columns incl the ones column
SCALE = 1.0 / np.sqrt(DK)

# t1 pipeline blocks: (offset, width)
BLOCKS = [(0, 384), (384, 512), (896, 128)]


def emit(ctx: ExitStack, tc, io):
    nc = tc.nc

    xT, xTo, xo = io["xT"], io["xTo"], io["xo"]
    wq, wk, wv, wo, w1, w2 = io["wq"], io["wk"], io["wv"], io["wo"], io["w1"], io["w2"]
    out = io["out"]

    const = ctx.enter_context(tc.tile_pool(name="const", bufs=1))
    persist = ctx.enter_context(tc.tile_pool(name="persist", bufs=1))
    exp_pool = ctx.enter_context(tc.tile_pool(name="exp", bufs=3))
    cxu_pool = ctx.enter_context(tc.tile_pool(name="cxu", bufs=6))
    work = ctx.enter_context(tc.tile_pool(name="work", bufs=2))
    stat = ctx.enter_context(tc.tile_pool(name="stat", bufs=4))
    norm = ctx.enter_context(tc.tile_pool(name="norm", bufs=2))
    zpool = ctx.enter_context(tc.tile_pool(name="zpool", bufs=1))
    out_pool = ctx.enter_context(tc.tile_pool(name="out", bufs=2))

    mm_ps = ctx.enter_context(tc.tile_pool(name="mm_ps", bufs=2, space="PSUM"))
    sc_ps = ctx.enter_context(tc.tile_pool(name="sc_ps", bufs=2, space="PSUM"))
    ctx_ps = ctx.enter_context(tc.tile_pool(name="ctx_ps", bufs=1, space="PSUM"))

    # ---- persistent SBUF arrays ----
    wk_sb = persist.tile([P, KD, D], F8, tag="wk")
    nc.sync.dma_start(wk_sb[:], wk[:, :])
    xT_sb = persist.tile([P, KD, S], F8, tag="xT")
    for k in range(KD):
        nc.sync.dma_start(
            xT_sb[:, k, :], xT[:, :].rearrange("(k p) t -> p k t", p=P)[:, k, :]
        )
    wq_sb = persist.tile([P, KD, D], F8, tag="wq")
    nc.sync.dma_start(wq_sb[:], wq[:, :])
    xTo_sb = persist.tile([P, KD, T1], F8, tag="xTo")
    for k in range(KD):
        nc.sync.dma_start(
            xTo_sb[:, k, :], xTo[:, :].rearrange("(k p) t -> p k t", p=P)[:, k, :]
        )
    wv_sb = persist.tile([P, KD, D], F8, tag="wv")
    nc.sync.dma_start(wv_sb[:], wv[:, :])
    wo_sb = persist.tile([P, KD, D], F8, tag="wo")
    nc.sync.dma_start(wo_sb[:], wo[:, :])

    kt_sb = persist.tile([P, NPAIR, S], BF16, tag="kt")
    qt_sb = persist.tile([P, NPAIR, T1], BF16, tag="qt")
    # V per head with ones column at 64: [t2 128, t2tile, head, 65] (bf16)
    ve_sb = persist.tile([P, NT2, H, DV1], BF16, tag="ve")
    nc.vector.memset(ve_sb[:, :, :, DK:DV1], 1.0)
    ctxT_sb = persist.tile([P, NPAIR, T1], F8, tag="ctxT")
    x1_sb = persist.tile([P, NT1, D], BF16, tag="x1")
    x1T_sb = persist.tile([P, KD, T1], BF16, tag="x1T")
    h1T_sb = persist.tile([P, NDFF, T1], BF16, tag="h1T")
    w1_sb = persist.tile([P, KD, DFF], BF16, tag="w1")
    w2_sb = persist.tile([P, NDFF, D], BF16, tag="w2")
    xo_sb = persist.tile([P, NT1, D], BF16, tag="xo")

    # ---- constants ----
    ident_sb = const.tile([P, P], BF16)
    make_identity(nc, ident_sb[:])
    eps_sb = const.tile([P, 1], F32)
    nc.vector.memset(eps_sb[:], EPS)
    # indicator for the 1/Z partition-broadcast: row 0 -> partitions 0:64
    # (even head), row 1 -> partitions 64:128 (odd head)
    ind2_sb = const.tile([2, P], BF16)
    nc.gpsimd.dma_start(ind2_sb[:], io["ind2"][:, :])

    # per-partition bias tiles (feature-major evictions)
    bqt = const.tile([P, KD], F32)
    nc.gpsimd.dma_start(bqt[:], io["bq"][:].rearrange("(m p) -> p m", p=P))
    bkt = const.tile([P, KD], F32)
    nc.gpsimd.dma_start(bkt[:], io["bk"][:].rearrange("(m p) -> p m", p=P))
    b1t = const.tile([P, NDFF], F32)
    nc.gpsimd.dma_start(b1t[:], io["b1"][:].rearrange("(m p) -> p m", p=P))

    # free-axis broadcast tiles (token-major ops)
    def bc_tile(name):
        t = const.tile([P, D], BF16, tag=f"bc_{name}")
        a = io[name][:]
        bcast = bass.AP(tensor=a.tensor, offset=a.offset, ap=[[0, P]] + list(a.ap))
        nc.gpsimd.dma_start(t[:], bcast)
        return t

    bvb = bc_tile("bv")
    b2b = bc_tile("b2")
    g1b = bc_tile("g1")
    be1b = bc_tile("be1")
    g2b = bc_tile("g2")
    be2b = bc_tile("be2")

    # big prefetches ride the (otherwise idle) gpsimd DMA queue, AFTER the
    # consts above -- a DMA trigger blocks its issuing engine's FIFO while
    # the queue is full.
    nc.gpsimd.dma_start(w1_sb[:], w1[:, :])
    nc.gpsimd.dma_start(w2_sb[:], w2[:, :])
    # residual (token-major, bf16, bo pre-added on host) preloaded once
    for t in range(NT1):
        nc.gpsimd.dma_start(xo_sb[:, t, :], xo[t * P:(t + 1) * P, :])

    # ---- projections (fp8 DoubleRow, contract 512 = 2 passes) ----
    def kproj(m, nb):
        ps = mm_ps.tile([P, 512], F32, tag="mm")
        for j in range(2):
            nc.tensor.matmul(
                ps[:],
                wk_sb[:, 2 * j:2 * j + 2, m * P:(m + 1) * P],
                xT_sb[:, 2 * j:2 * j + 2, nb * 512:(nb + 1) * 512],
                start=(j == 0), stop=(j == 1), perf_mode=DR,
            )
        nc.scalar.activation(
            kt_sb[:, m, nb * 512:(nb + 1) * 512], ps[:], AF.Identity,
            bias=bkt[:, m:m + 1],
        )

    def qproj(m, nb):
        ps = mm_ps.tile([P, 512], F32, tag="mm")
        for j in range(2):
            nc.tensor.matmul(
                ps[:],
                wq_sb[:, 2 * j:2 * j + 2, m * P:(m + 1) * P],
                xTo_sb[:, 2 * j:2 * j + 2, nb * 512:(nb + 1) * 512],
                start=(j == 0), stop=(j == 1), perf_mode=DR,
            )
        nc.scalar.activation(
            qt_sb[:, m, nb * 512:(nb + 1) * 512], ps[:], AF.Identity,
            bias=bqt[:, m:m + 1],
        )

    def vproj(i):
        ps = mm_ps.tile([P, 512], F32, tag="mm")
        for j in range(2):
            nc.tensor.matmul(
                ps[:],
                xT_sb[:, 2 * j:2 * j + 2, i * P:(i + 1) * P],
                wv_sb[:, 2 * j:2 * j + 2, :],
                start=(j == 0), stop=(j == 1), perf_mode=DR,
            )
        nc.vector.tensor_tensor(
            ve_sb[:, i, :, 0:DK],
            ps[:].rearrange("p (h d) -> p h d", h=H),
            bvb[:].rearrange("p (h d) -> p h d", h=H),
            OP.add,
        )

    # ---- post-attention / FFN stages ----
    post_stats = {}

    def post_attn1(t1t, eng):
        """Wo (fp8 DR) + residual (bo pre-folded into xo) + bn stats."""
        ao = mm_ps.tile([P, 512], F32, tag="mm")
        for j in range(2):
            nc.tensor.matmul(
                ao[:],
                ctxT_sb[:, 2 * j:2 * j + 2, t1t * P:(t1t + 1) * P],
                wo_sb[:, 2 * j:2 * j + 2, :],
                start=(j == 0), stop=(j == 1), perf_mode=DR,
            )
        rslot = x1_sb[:, t1t, :]
        nc.vector.tensor_tensor(rslot, ao[:], xo_sb[:, t1t, :], OP.add)
        st = stat.tile([P, 6], F32, tag="st")
        nc.vector.bn_stats(st[:], rslot)
        mv = stat.tile([P, 2], F32, tag="mv")
        nc.vector.bn_aggr(mv[:], st[:])
        post_stats[t1t] = [mv, None]

    def post_rstd(t1t):
        """LN1 rstd on ACT (its DVE inputs are a stage old)."""
        mv = post_stats[t1t][0]
        lnv = stat.tile([P, 1], F32, tag="lnv")
        nc.scalar.activation(lnv[:], mv[:, 1:2], AF.Ln, bias=eps_sb[:, 0:1])
        rstd = stat.tile([P, 1], F32, tag="rstd")
        nc.scalar.activation(rstd[:], lnv[:], AF.Exp, scale=-0.5)
        post_stats[t1t][1] = rstd

    def post_attn2(t1t, eng):
        """LN1 normalize+affine (in the x1 slot) + transpose(x1) -> x1T."""
        mv, rstd = post_stats.pop(t1t)
        rslot = x1_sb[:, t1t, :]
        xc = work.tile([P, D], F32, tag="xc")
        nc.vector.tensor_scalar(
            xc[:], rslot, mv[:, 0:1], rstd[:], op0=OP.subtract, op1=OP.mult
        )
        xg = work.tile([P, D], F32, tag="xg")
        eng.tensor_tensor(xg[:], xc[:], g1b[:], OP.mult)
        eng.tensor_tensor(rslot, xg[:], be1b[:], OP.add)
        for j in range(KD):
            tp = mm_ps.tile([P, P], BF16, tag="mm")
            nc.tensor.transpose(
                tp[:], x1_sb[:, t1t, j * P:(j + 1) * P], ident_sb[:]
            )
            if eng is nc.vector:
                nc.scalar.copy(x1T_sb[:, j, t1t * P:(t1t + 1) * P], tp[:])
            else:
                nc.vector.tensor_copy(x1T_sb[:, j, t1t * P:(t1t + 1) * P], tp[:])

    def ffn1(lo, tb, m0, m1, on_act):
        for m in range(m0, m1):
            ps = mm_ps.tile([P, 512], F32, tag="mm")
            for k in range(KD):
                nc.tensor.matmul(
                    ps[:, 0:tb],
                    w1_sb[:, k, m * P:(m + 1) * P],
                    x1T_sb[:, k, lo:lo + tb],
                    start=(k == 0), stop=(k == KD - 1),
                )
            if on_act:
                nc.scalar.activation(
                    h1T_sb[:, m, lo:lo + tb], ps[:, 0:tb], AF.Relu,
                    bias=b1t[:, m:m + 1],
                )
            else:
                nc.vector.tensor_scalar(
                    h1T_sb[:, m, lo:lo + tb], ps[:, 0:tb],
                    b1t[:, m:m + 1], 0.0, op0=OP.add, op1=OP.max,
                )

    ffn2_stats = {}

    def ffn2a(t1t, eng):
        """FFN2 matmuls + residual + LN2 stats (no ACT ops)."""
        ff = mm_ps.tile([P, 512], F32, tag="mm")
        for k in range(NDFF):
            nc.tensor.matmul(
                ff[:],
                h1T_sb[:, k, t1t * P:(t1t + 1) * P],
                w2_sb[:, k, :],
                start=(k == 0), stop=(k == NDFF - 1),
            )
        r = work.tile([P, D], F32, tag="r2")
        nc.vector.tensor_tensor(r[:], ff[:], x1_sb[:, t1t, :], OP.add)
        nc.vector.tensor_tensor(r[:], r[:], b2b[:], OP.add)
        st = stat.tile([P, 6], F32, tag="st")
        nc.vector.bn_stats(st[:], r[:])
        mv = stat.tile([P, 2], F32, tag="mv")
        nc.vector.bn_aggr(mv[:], st[:])
        ffn2_stats[t1t] = (r, mv)

    def ffn2b(t1t, eng):
        """LN2 rstd (ACT; stats a stage old) + normalize + store."""
        r, mv = ffn2_stats.pop(t1t)
        lnv = stat.tile([P, 1], F32, tag="lnv")
        nc.scalar.activation(lnv[:], mv[:, 1:2], AF.Ln, bias=eps_sb[:, 0:1])
        rstd = stat.tile([P, 1], F32, tag="rstd")
        nc.scalar.activation(rstd[:], lnv[:], AF.Exp, scale=-0.5)
        xc = work.tile([P, D], F32, tag="xc")
        nc.vector.tensor_scalar(
            xc[:], r[:], mv[:, 0:1], rstd[:], op0=OP.subtract, op1=OP.mult
        )
        o = out_pool.tile([P, D], F32)
        xg = work.tile([P, D], F32, tag="xg")
        eng.tensor_tensor(xg[:], xc[:], g2b[:], OP.mult)
        eng.tensor_tensor(o[:], xg[:], be2b[:], OP.add)
        nc.sync.dma_start(out[t1t * P:(t1t + 1) * P, :], o[:])

    # ---- attention ----
    def normalize_pair(pair, t1s, tb, zall, cxu):
        """1/Z for one head pair: [2,tb] ln/exp + ONE 2-contract broadcast
        matmul (even head -> partitions 0:64, odd -> 64:128)."""
        hA, hB = 2 * pair, 2 * pair + 1
        zs = slice(pair * 512, pair * 512 + tb)
        lz = norm.tile([2, 512], F32, tag="lz")
        nc.scalar.activation(lz[:, 0:tb], zall[0:2, zs], AF.Ln)
        rz = norm.tile([2, 512], BF16, tag="rz")
        nc.scalar.activation(rz[:, 0:tb], lz[:, 0:tb], AF.Exp, scale=-1.0)
        bch = mm_ps.tile([P, 512], F32, tag="mm")
        nc.tensor.matmul(
            bch[:, 0:tb], ind2_sb[:, :], rz[:, 0:tb], start=True, stop=True,
        )
        nc.vector.tensor_tensor(
            ctxT_sb[0:64, pair, t1s], cxu.pop(hA)[:, 0:tb], bch[0:64, 0:tb],
            OP.mult,
        )
        stg = work.tile([64, 512], F8, tag="stg")
        nc.vector.tensor_tensor(
            stg[:, 0:tb], cxu.pop(hB)[:, 0:tb], bch[64:128, 0:tb], OP.mult
        )
        nc.sync.dma_start(ctxT_sb[64:128, pair, t1s], stg[:, 0:tb])

    def attention_block(b, slot):
        """slot(b, pair, half) is called twice per pair (mid-pair, pair
        end) to emit one hidden-work stage as a dense burst."""
        lo, tb = BLOCKS[b]
        t1s = slice(lo, lo + tb)
        zall = zpool.tile([2, NPAIR * 512], F32, tag="zall")
        cxu = {}
        for pair in range(NPAIR):
            hA, hB = 2 * pair, 2 * pair + 1
            cxA = ctx_ps.tile([DV1, 512], F32, tag="cxA")
            cxB = ctx_ps.tile([DV1, 512], F32, tag="cxB")
            for t2 in range(NT2):
                if t2 == 2 and pair > 0:
                    # previous pair's Z rows have landed by now
                    normalize_pair(pair - 1, t1s, tb, zall, cxu)
                sp = sc_ps.tile([P, 2, 512], F32, tag="s")
                for idx, hrow in enumerate((0, 64)):
                    nc.tensor.matmul(
                        sp[:, idx, 0:tb],
                        kt_sb[hrow:hrow + 64, pair, t2 * P:(t2 + 1) * P],
                        qt_sb[hrow:hrow + 64, pair, t1s],
                        start=True, stop=True, tile_position=(hrow, 0),
                        skip_group_check=(idx > 0),
                    )
                e = exp_pool.tile([P, 2, 512], BF16, tag="e")
                nc.scalar.activation(
                    e[:, :, 0:tb], sp[:, :, 0:tb], AF.Exp, scale=SCALE
                )
                first, last = t2 == 0, t2 == NT2 - 1
                nc.tensor.matmul(
                    cxA[:, 0:tb], ve_sb[:, t2, hA, :], e[:, 0, 0:tb],
                    start=first, stop=last,
                )
                nc.tensor.matmul(
                    cxB[:, 0:tb], ve_sb[:, t2, hB, :], e[:, 1, 0:tb],
                    start=first, stop=last,
                )
                if t2 == NT2 // 2:
                    slot(b, pair, 0)
            # evict unnormalized ctx (bf16) and gather Z rows (f32)
            for h, cx in ((hA, cxA), (hB, cxB)):
                cu = cxu_pool.tile([64, 512], BF16, tag="cu")
                nc.vector.tensor_copy(cu[:, 0:tb], cx[0:64, 0:tb])
                zst = norm.tile([P, 512], F32, tag="zst")
                nc.vector.tensor_copy(zst[64:65, 0:tb], cx[64:65, 0:tb])
                nc.sync.dma_start(
                    zall[h & 1:(h & 1) + 1, pair * 512:pair * 512 + tb],
                    zst[64:65, 0:tb],
                )
                cxu[h] = cu
            slot(b, pair, 1)
        normalize_pair(NPAIR - 1, t1s, tb, zall, cxu)

    # hidden-work stages for a finished block, run as dense bursts in the
    # next block's 8 slots (2 per pair)
    def make_stages(b):
        lo, tb = BLOCKS[b]
        hidden = b < len(BLOCKS) - 1
        eng = nc.gpsimd if hidden else nc.vector
        tiles = [lo // P + i for i in range(tb // P)]
        half = (len(tiles) + 1) // 2

        def s0():
            for t in tiles[:half]:
                post_attn1(t, eng)

        def s1():
            for t in tiles[half:]:
                post_attn1(t, eng)
            for t in tiles:
                post_rstd(t)

        def s2():
            for t in tiles[:half]:
                post_attn2(t, eng)

        def s3():
            for t in tiles[half:]:
                post_attn2(t, eng)

        def s4():
            ffn1(lo, tb, 0, NDFF // 2, on_act=not hidden)

        def s5():
            ffn1(lo, tb, NDFF // 2, NDFF, on_act=not hidden)

        def s6():
            # a/b paired two tiles at a time: keeps <=2 r tiles in flight
            # (work pool bufs=2; more would WAR-deadlock the DVE FIFO)
            for t in tiles[:2]:
                ffn2a(t, eng)
            for t in tiles[:2]:
                ffn2b(t, eng)

        def s7():
            for t in tiles[2:]:
                ffn2a(t, eng)
            for t in tiles[2:]:
                ffn2b(t, eng)

        return [s0, s1, s2, s3, s4, s5, s6, s7]

    pending = {}

    def slot(b, pair, half):
        idx = 2 * pair + half
        stages = pending.get(b - 1)
        if stages and idx < len(stages):
            stages[idx]()

    # startup: all projections upfront as one dense burst (fp8 DMA is
    # cheap; PE warms up on them)
    for m in range(NPAIR):
        for nb in range(4):
            kproj(m, nb)
    for m in range(NPAIR):
        for nb in range(2):
            qproj(m, nb)
    for i in range(NT2):
        vproj(i)

    for b in range(len(BLOCKS)):
        attention_block(b, slot)
        pending[b] = make_stages(b)
    # tail: last block's stages, plus anything unfinished
    for st in pending[len(BLOCKS) - 1]:
        st()


def _patch_act_tables():
    """Force every ACT op onto the natural_log_exp_and_others table set so
    the kernel pays one ACT_TABLE_LOAD instead of thrashing between the
    per-function default sets."""
    import functools
    import concourse.hw_specs as hw_specs

    if getattr(hw_specs, "_nle_only", False):
        return
    orig = hw_specs.get_activation_tables

    @functools.cache
    def nle_only(arch):
        tabs = orig(arch)
        return {
            k: (v if k == "natural_log_exp_and_others" else set())
            for k, v in tabs.items()
        }

    hw_specs.get_activation_tables = nle_only
    hw_specs._nle_only = True
    if getattr(bacc, "get_activation_tables", None) is not None:
        bacc.get_activation_tables = nle_only


def build_program():
    _patch_act_tables()
    nc = bacc.Bacc("TRN2", target_bir_lowering=False, debug=False, num_devices=NCORES)
    io = {}
    io["xT"] = nc.dram_tensor("xT", [D, S], F8, kind="ExternalInput").ap()
    io["xTo"] = nc.dram_tensor("xTo", [D, T1], F8, kind="ExternalInput").ap()
    io["xo"] = nc.dram_tensor("xo", [T1, D], BF16, kind="ExternalInput").ap()
    for name, shape, dt in [
        ("wq", [P, D * D // P], F8), ("wk", [P, D * D // P], F8),
        ("wv", [P, D * D // P], F8), ("wo", [P, D * D // P], F8),
        ("w1", [P, D * DFF // P], BF16), ("w2", [P, DFF * D // P], BF16),
    ]:
        io[name] = nc.dram_tensor(name, shape, dt, kind="ExternalInput").ap()
    for name, n in [
        ("bq", D), ("bk", D), ("bv", D), ("bo", D), ("b1", DFF), ("b2", D),
        ("g1", D), ("be1", D), ("g2", D), ("be2", D),
    ]:
        io[name] = nc.dram_tensor(name, [n], F32, kind="ExternalInput").ap()
    io["ind2"] = nc.dram_tensor("ind2", [2, P], BF16, kind="ExternalInput").ap()
    io["out"] = nc.dram_tensor("out", [T1, D], F32, kind="ExternalOutput").ap()

    with tile.TileContext(nc) as tc:
        with ExitStack() as ctx:
            emit(ctx, tc, io)
    nc.compile()
    return nc


def make_in_maps(x, Wq, bq, Wk, bk, Wv, bv, Wo, bo, W1, b1, W2, b2,
                 g1, be1, g2, be2):
    bf = ml_dtypes.bfloat16
    f8 = ml_dtypes.float8_e4m3fn
    f32 = np.float32
    def swz(w, dt):
        # device sbuf layout [p, k, m] for a [k*128+p, m] weight
        w = np.asarray(w, f32)
        kd, m = w.shape[0] // P, w.shape[1]
        return np.ascontiguousarray(
            w.reshape(kd, P, m).transpose(1, 0, 2).reshape(P, kd * m)
        ).astype(dt)

    shared = {
        "wq": swz(Wq, f8),
        "wk": swz(Wk, f8),
        "wv": swz(Wv, f8),
        "wo": swz(Wo, f8),
        "w1": swz(W1, bf),
        "w2": swz(W2, bf),
        "bq": np.asarray(bq, f32),
        "bk": np.asarray(bk, f32), "bv": np.asarray(bv, f32),
        "bo": np.asarray(bo, f32), "b1": np.asarray(b1, f32),
        "b2": np.asarray(b2, f32), "g1": np.asarray(g1, f32),
        "be1": np.asarray(be1, f32), "g2": np.asarray(g2, f32),
        "be2": np.asarray(be2, f32),
        "ind2": np.kron(np.eye(2, dtype=f32), np.ones((1, DK), f32)).astype(bf),
    }
    x = np.asarray(x, f32)
    in_maps = []
    for c in range(NCORES):
        b, half = divmod(c, 2)
        xb = x[b]                                    # [S, D] f32
        xTb = np.ascontiguousarray(xb.T).astype(f8)  # [D, S] fp8
        sl = slice(half * T1, (half + 1) * T1)
        m = dict(shared)
        m["xT"] = xTb
        m["xTo"] = np.ascontiguousarray(xTb[:, sl])
        # bo folded into the residual on the host
        m["xo"] = (xb[sl] + np.asarray(bo, f32)[None, :]).astype(bf)
        in_maps.append(m)
    return in_maps


_prog_cache = {}


def get_program():
    if "nc" not in _prog_cache:
        _prog_cache["nc"] = build_program()
    return _prog_cache["nc"]


def kernel(**inputs) -> np.ndarray:
    nc = get_program()
    in_maps = make_in_maps(**inputs)
    res = run_bass_kernel_spmd(nc, in_maps, core_ids=list(range(NCORES)))
    out = np.empty((B, S, D), np.float32)
    for c in range(NCORES):
        b, half = divmod(c, 2)
        out[b, half * T1:(half + 1) * T1] = res.results[c]["out"]
    return out


if __name__ == "__main__":
    print("building program...")
    get_program()
    print("built")
